# revision 1
# baseline (speedup 1.0000x reference)
"""MCRNN (multi-compartment spiking RNN) Trainium2 kernel.

Reference computation (per batch element, data-parallel over B across 8 cores):
  combined = concat([inputs, state0], -1)                      [T,B,IN+H]
  apical = popnorm(combined @ Wa^T + ba) ; basal = popnorm(.. Wb ..)
  soma   = popnorm(inputs  @ Ws^T + bs)
  scan over T: dend = sigmoid(a)*tanh(b); mem += (s+dend-mem)/2;
               spk = mem>0.5; mem *= 1-spk

Kernel strategy (per core, B_shard=64, tokens=(t,b) t-major, 16 m-tiles of 128):
  - matmuls on PE in bf16 hi+lo split: X is 0/1 so X@W_hi products are exact;
    W_hi+W_lo reconstructs fp32 weights to ~2^-18 relative. Bias added via two
    ones-rows in X paired with b_hi/b_lo weight rows (K=2 chunk).
  - popnorm stats via bn_stats/bn_aggr from PSUM; normalize fused into the
    sigmoid/tanh activations via per-token scale/bias (ACT reads PSUM).
  - membrane scan: state w = v*keep (x2-scaled membrane), per step:
      v = 0.5*w + u ; spk = v > 1 ; w' = (v<=1)*v
    all on fused scalar_tensor_tensor ops; t1 drive shifted to partitions 0-63
    by an SBUF->SBUF DMA.
Output spikes written as bf16 (exact 0/1), host converts to fp32.
"""
import numpy as np
import ml_dtypes

import concourse.bass as bass
import concourse.bacc as bacc
import concourse.mybir as mybir
from concourse.tile import TileContext
from concourse.bass_utils import run_bass_kernel_spmd

F = mybir.dt.float32
BF = mybir.dt.bfloat16
AF = mybir.ActivationFunctionType
OP = mybir.AluOpType

T, B, IN, H = 32, 512, 1024, 1024
K = IN + H
NCORES = 8
BS = B // NCORES          # 64 batch per core
M_TOK = T * BS            # 2048 tokens per core
MT = M_TOK // 128         # 16 m-tiles
KC = K // 128             # 16 k-chunks (soma uses first 8)
KCS = IN // 128
HH = H // 2
TAU, VTH, EPS = 2.0, 0.5, 1e-5
THR = 2.0 * VTH           # spike threshold in v units (v = 2*mem)


def _bf(x):
    return np.asarray(x.astype(ml_dtypes.bfloat16))


def _wsplit(w, b):
    """W [H, Kw] fp32 -> (hi, lo) tiled [kc,128,H] bf16 + bias rows [2,H] bf16."""
    wt = np.ascontiguousarray(w.T).astype(np.float32)        # [Kw, H]
    hi = wt.astype(ml_dtypes.bfloat16)
    lo = (wt - hi.astype(np.float32)).astype(ml_dtypes.bfloat16)
    kc = wt.shape[0] // 128
    bh = b.astype(ml_dtypes.bfloat16)
    bl = (b.astype(np.float32) - bh.astype(np.float32)).astype(ml_dtypes.bfloat16)
    brow = np.ascontiguousarray(np.stack([bh, bl]))
    return (np.ascontiguousarray(np.asarray(hi).reshape(kc, 128, H)),
            np.ascontiguousarray(np.asarray(lo).reshape(kc, 128, H)),
            np.asarray(brow))


def _build(identity_affine: bool):
    """Build the SPMD single-core program. Returns finalized nc."""
    nc = bacc.Bacc("TRN2", target_bir_lowering=False, debug=False)

    xt_d = nc.dram_tensor("xt", [(KC + 1) * 128, M_TOK], BF, kind="ExternalInput").ap()
    d = {}
    wspec = [("wah", KC), ("wal", KC), ("wbh", KC), ("wbl", KC), ("wsh", KCS), ("wsl", KCS)]
    for nm, kc in wspec:
        d[nm] = nc.dram_tensor(nm, [kc, 128, H], BF, kind="ExternalInput").ap()
    for nm in ("bar", "bbr", "bsr"):
        d[nm] = nc.dram_tensor(nm, [2, H], BF, kind="ExternalInput").ap()
    if not identity_affine:
        # replicated gamma / beta per stage (fp32)
        for nm in ("gar", "btar", "gbr", "btbr", "gsr", "btsr"):
            d[nm] = nc.dram_tensor(nm, [128, H], F, kind="ExternalInput").ap()
    spk_d = nc.dram_tensor("spk", [M_TOK, H], BF, kind="ExternalOutput").ap()

    with TileContext(nc) as tc:
        with tc.tile_pool(name="w", bufs=1) as wp, \
             tc.tile_pool(name="x", bufs=2) as xp, \
             tc.tile_pool(name="z", bufs=1) as zp, \
             tc.tile_pool(name="st", bufs=1) as stp, \
             tc.tile_pool(name="ps", bufs=1, space="PSUM") as ps:

            # ---- resident weights ----
            w_s = {}
            for nm, kc in wspec:
                w_s[nm] = wp.tile([128, kc, H], BF, name=f"t_{nm}")
                for c in range(kc):
                    nc.sync.dma_start(w_s[nm][:, c, :], d[nm][c])
            for nm in ("bar", "bbr", "bsr"):
                w_s[nm] = wp.tile([2, H], BF, name=f"t_{nm}")
                nc.sync.dma_start(w_s[nm][:], d[nm])
            if not identity_affine:
                for nm in ("gar", "btar", "gbr", "btbr", "gsr", "btsr"):
                    w_s[nm] = wp.tile([128, H], F, name=f"t_{nm}")
                    nc.sync.dma_start(w_s[nm][:], d[nm])

            # ---- scan state (x2-scaled, per h-half), zero-init ----
            w_cur = {}
            for hh in range(2):
                w_cur[hh] = stp.tile([64, HH], F, tag=f"wst{hh}", bufs=2,
                                     name=f"w_init{hh}")
                nc.vector.memset(w_cur[hh][:], 0.0)

            for m in range(MT):
                # ---- stream X^T chunks for this m-tile ----
                xt = xp.tile([128, KC + 1, 128], BF, tag="xt", name=f"xt_{m}")
                for c in range(KC + 1):
                    nc.sync.dma_start(xt[:, c, :],
                                      xt_d[c * 128:(c + 1) * 128,
                                           m * 128:(m + 1) * 128])
                ones2 = xt[0:2, KC, :]

                # ---- matmuls: stage-major, k-contiguous, bias rows last ----
                pa = ps.tile([128, H], F, tag="pa", name=f"pa_{m}")
                pb = ps.tile([128, H], F, tag="pb", name=f"pb_{m}")
                psm = ps.tile([128, H], F, tag="psm", name=f"psm_{m}")
                for pt, kcn, hi, lo, br in ((pa, KC, "wah", "wal", "bar"),
                                            (pb, KC, "wbh", "wbl", "bbr"),
                                            (psm, KCS, "wsh", "wsl", "bsr")):
                    for n in range(2):
                        sl = slice(n * 512, (n + 1) * 512)
                        for k in range(kcn):
                            lhsT = xt[:, k, :]
                            nc.tensor.matmul(pt[:, sl], lhsT=lhsT,
                                             rhs=w_s[hi][:, k, sl],
                                             start=(k == 0), stop=False)
                            nc.tensor.matmul(pt[:, sl], lhsT=lhsT,
                                             rhs=w_s[lo][:, k, sl],
                                             start=False, stop=False)
                        nc.tensor.matmul(pt[:, sl], lhsT=ones2,
                                         rhs=w_s[br][:, sl],
                                         start=False, stop=True)

                # ---- popnorm stats from PSUM ----
                stats = stp.tile([128, 3, 2, 6], F, tag="stats", bufs=2,
                                 name=f"stats_{m}")
                for i, pt in enumerate((pa, pb, psm)):
                    nc.vector.bn_stats(stats[:, i, 0, :], pt[:, 0:512])
                    nc.vector.bn_stats(stats[:, i, 1, :], pt[:, 512:1024])
                agg = stp.tile([128, 8], F, tag="agg", bufs=2, name=f"agg_{m}")
                aggr = agg[:, 0:6].rearrange("p (i t) -> p i t", i=3)
                for i in range(3):
                    nc.vector.bn_aggr(aggr[:, i, :],
                                      stats[:, i, :, :].rearrange("p c s -> p (c s)"))
                nc.vector.memset(agg[:, 6:7], EPS)
                std = stp.tile([128, 3], F, tag="std", bufs=2, name=f"std_{m}")
                nc.scalar.activation(std[:], aggr[:, :, 1], AF.Sqrt, bias=agg[:, 6:7])
                rn = stp.tile([128, 8], F, tag="rn", bufs=2, name=f"rn_{m}")
                nc.vector.reciprocal(rn[:, 0:3], std[:])
                nc.vector.scalar_tensor_tensor(rn[:, 3:6], aggr[:, :, 0], -1.0,
                                               rn[:, 0:3], OP.mult, OP.mult)

                # ---- per h-half: normalize+nonlinearity, drive u, scan ----
                for hh in range(2):
                    hsl = slice(hh * HH, (hh + 1) * HH)
                    sa = zp.tile([128, HH], F, tag="sa", name=f"sa_{m}_{hh}")
                    tb = zp.tile([128, HH], F, tag="tb", name=f"tb_{m}_{hh}")
                    sn = zp.tile([128, HH], F, tag="sn", name=f"sn_{m}_{hh}")
                    if identity_affine:
                        nc.scalar.activation(sa[:], pa[:, hsl], AF.Sigmoid,
                                             scale=rn[:, 0:1], bias=rn[:, 3:4])
                        nc.scalar.activation(tb[:], pb[:, hsl], AF.Tanh,
                                             scale=rn[:, 1:2], bias=rn[:, 4:5])
                        nc.scalar.activation(sn[:], psm[:, hsl], AF.Identity,
                                             scale=rn[:, 2:3], bias=rn[:, 5:6])
                    else:
                        for (pt, i, g, bt, outt, fn) in (
                                (pa, 0, "gar", "btar", sa, AF.Sigmoid),
                                (pb, 1, "gbr", "btbr", tb, AF.Tanh),
                                (psm, 2, "gsr", "btsr", sn, AF.Identity)):
                            y = zp.tile([128, HH], F, tag="y", name=f"y_{m}_{hh}_{i}")
                            nc.scalar.activation(y[:], pt[:, hsl], AF.Identity,
                                                 scale=rn[:, i:i+1], bias=rn[:, 3+i:4+i])
                            yg = zp.tile([128, HH], F, tag="yg", name=f"yg_{m}_{hh}_{i}")
                            nc.vector.scalar_tensor_tensor(
                                yg[:], y[:], 0.0, w_s[g][:, hsl], OP.bypass, OP.mult)
                            nc.vector.scalar_tensor_tensor(
                                yg[:], yg[:], 0.0, w_s[bt][:, hsl], OP.bypass, OP.add)
                            nc.scalar.activation(outt[:], yg[:], fn)

                    dend = zp.tile([128, HH], F, tag="dend", name=f"dend_{m}_{hh}")
                    nc.vector.tensor_tensor(dend[:], sa[:], tb[:], OP.mult)
                    u = zp.tile([128, HH], F, tag="u", bufs=2, name=f"u_{m}_{hh}")
                    nc.vector.tensor_tensor(u[:], dend[:], sn[:], OP.add)

                    # shift the t1 drive down to partitions 0-63
                    u1 = stp.tile([64, HH], F, tag="u1", bufs=2, name=f"u1_{m}_{hh}")
                    nc.sync.dma_start(u1[:], u[64:128, :])

                    # scan step t0 = 2m
                    v0 = stp.tile([64, HH], F, tag="v", bufs=2, name=f"v0_{m}_{hh}")
                    nc.vector.scalar_tensor_tensor(v0[:], w_cur[hh][:], 0.5,
                                                   u[0:64, :], OP.mult, OP.add)
                    spk0 = stp.tile([64, HH], BF, tag="spk", bufs=2,
                                    name=f"spk0_{m}_{hh}")
                    nc.vector.tensor_single_scalar(spk0[:], v0[:], THR, OP.is_gt)
                    w0 = stp.tile([64, HH], F, tag=f"wst{hh}", bufs=2,
                                  name=f"w0_{m}_{hh}")
                    nc.vector.scalar_tensor_tensor(w0[:], v0[:], THR, v0[:],
                                                   OP.is_le, OP.mult)

                    # scan step t1 = 2m+1
                    v1 = stp.tile([64, HH], F, tag="v", bufs=2, name=f"v1_{m}_{hh}")
                    nc.vector.scalar_tensor_tensor(v1[:], w0[:], 0.5,
                                                   u1[:], OP.mult, OP.add)
                    spk1 = stp.tile([64, HH], BF, tag="spk", bufs=2,
                                    name=f"spk1_{m}_{hh}")
                    nc.vector.tensor_single_scalar(spk1[:], v1[:], THR, OP.is_gt)
                    w1 = stp.tile([64, HH], F, tag=f"wst{hh}", bufs=2,
                                  name=f"w1_{m}_{hh}")
                    nc.vector.scalar_tensor_tensor(w1[:], v1[:], THR, v1[:],
                                                   OP.is_le, OP.mult)
                    w_cur[hh] = w1

                    nc.sync.dma_start(spk_d[m * 128:m * 128 + 64, hsl], spk0[:])
                    nc.sync.dma_start(spk_d[m * 128 + 64:(m + 1) * 128, hsl], spk1[:])

    nc.finalize()
    return nc


_CACHE = {}


def kernel(inputs, state0, Wa, ba, Wb, bb, Ws, bs, ga, bta, gb, btb, gs, bts,
           **unused):
    inputs = np.asarray(inputs, np.float32)
    state0 = np.asarray(state0, np.float32)

    identity_affine = bool(
        np.all(ga == 1.0) and np.all(bta == 0.0) and
        np.all(gb == 1.0) and np.all(btb == 0.0) and
        np.all(gs == 1.0) and np.all(bts == 0.0))

    wah, wal, bar = _wsplit(np.asarray(Wa, np.float32), np.asarray(ba, np.float32))
    wbh, wbl, bbr = _wsplit(np.asarray(Wb, np.float32), np.asarray(bb, np.float32))
    wsh, wsl, bsr = _wsplit(np.asarray(Ws, np.float32), np.asarray(bs, np.float32))

    base = {"wah": wah, "wal": wal, "wbh": wbh, "wbl": wbl,
            "wsh": wsh, "wsl": wsl, "bar": bar, "bbr": bbr, "bsr": bsr}
    if not identity_affine:
        for nm, a in (("gar", ga), ("btar", bta), ("gbr", gb),
                      ("btbr", btb), ("gsr", gs), ("btsr", bts)):
            base[nm] = np.ascontiguousarray(
                np.broadcast_to(np.asarray(a, np.float32), (128, H)))

    # per-core X^T shard: [(KC+1)*128, 2048] bf16, ones rows at K..K+1
    comb = np.concatenate([inputs, state0], axis=-1)      # [T, B, K]
    in_maps = []
    for c in range(NCORES):
        xc = comb[:, c * BS:(c + 1) * BS, :].reshape(M_TOK, K)
        xt = np.empty(((KC + 1) * 128, M_TOK), np.float32)
        xt[:K] = xc.T
        xt[K:K + 2] = 1.0
        xt[K + 2:] = 0.0
        in_maps.append({**base, "xt": _bf(xt)})

    key = identity_affine
    if key not in _CACHE:
        _CACHE[key] = _build(identity_affine)
    nc = _CACHE[key]

    res = run_bass_kernel_spmd(nc, in_maps, core_ids=list(range(NCORES)))

    out = np.empty((T, B, H), np.float32)
    for c in range(NCORES):
        s = res.results[c]["spk"].astype(np.float32).reshape(T, BS, H)
        out[:, c * BS:(c + 1) * BS, :] = s
    return out
